# revision 17
# baseline (speedup 1.0000x reference)
"""Trainium2 Bass kernel for nn_FRPredictHeadWithFlatten.

Math restructuring (vs the reference):
  hat[w] = sb[w].T @ minv[w] @ sb[w]  (C x C) is never formed. With
  A = q @ sb.T (roi x 105, per-way 5-col blocks) and B[w] = A[w] @ minv[w]:
    Qbar.q = rho * rowsum(A*B),   |Qbar|^2 = rho^2*(dot - lam*rowsum(B*B))
  so scores = c1*rowsum(A*B) + c2*rowsum(B*B) + c3*|q|^2 with host scalars
  c1 = -exp(scale)*(rho^2-2rho)/C, c2 = exp(scale)*rho^2*lam/C,
  c3 = -exp(scale)/C (folded in as f32 per-partition scalars so they are
  never bf16-rounded; the selector matrices hold exact 0/1).
  The per-way row reductions run on the PE with selector matmuls so
  everything stays in transposed-activation layout. The 21 5x5 inverses are
  Gauss-Jordan in f32 on the vector engine in [way, 5*10] layout; the
  diagonal block extract/scatter goes through DRAM scratch (strided DRAM
  APs can address block diagonals in one DMA; SBUF APs cannot).

Sharding: data-parallel over roi (256 rows/core), weights + support/bg
replicated. All activations flow as X.T ([C on partitions, rows free]) so
every matmul has its contraction on partitions; weights are uploaded
pre-transposed and bf16-cast from the host (fp32 accumulate in PSUM; the
support output is additionally written back at f32 from PSUM).
"""

import sys

sys.path.insert(0, "/opt/trn_rl_repo")

import numpy as np
import ml_dtypes

import concourse.bacc as bacc
import concourse.tile as tile
from concourse.ap import AP
from concourse import mybir
from concourse.bass_utils import run_bass_kernel_spmd

WAY = 20
SHOT = 5
C = 1024
NCLS = 21
NC4 = NCLS * 4  # 84
ROI = 2048
N_CORES = 8
RPC = ROI // N_CORES  # 256 rois per core
W1 = WAY + 1  # 21 ways incl. background
SB = W1 * SHOT  # 105 sb rows
SBP = 112  # sb window incl. 7 junk cols (flow to zero through minv/E)
NSB2 = 200  # sup+bg columns
KT = C // 128  # 8 contraction tiles
NSUP = WAY * SHOT  # 100
NCOL = 2 * NSUP + RPC  # 456 act cols: [0:100 sup][100:200 bg][200:456 box]
# ZT col layout: [0:5 bgmean][5:105 s][105:205 bg][205:461 box]
ZOFF = 5
ZBOX = ZOFF + 2 * NSUP  # 205
ZCOLS = ZOFF + NCOL  # 461

F32 = mybir.dt.float32
BF16 = mybir.dt.bfloat16


def _emit(tc, nc, io, ctx):
    persist = ctx.enter_context(tc.tile_pool(name="persist", bufs=1))

    def _tile(shape, dtype, name):
        return persist.tile(shape, dtype, tag=name, name=name)

    xts, xtb, w1t, w2t, wbt = io["xts"], io["xtb"], io["w1t"], io["w2t"], io["wbt"]
    minz = io["minz"]
    b1, b2, bbt, lami, coef = io["b1"], io["b2"], io["bbt"], io["lami"], io["coef"]
    e1, e2, eq = io["e1"], io["e2"], io["eq"]
    scores_t, bbox_t, support_t = io["scores_t"], io["bbox_t"], io["support_t"]

    # ---- persistent SBUF tiles -------------------------------------------
    xts_a = _tile([128, KT * NSB2], BF16, name="xts_a")
    xtb_a = _tile([128, KT * RPC], BF16, name="xtb_a")
    w1_a = _tile([128, KT * C], BF16, name="w1_a")
    w2_a = _tile([128, KT * C], BF16, name="w2_a")
    wb_a = _tile([128, KT * NC4], BF16, name="wb_a")
    ht_k = [_tile([128, NCOL], BF16, name=f"ht{k}") for k in range(KT)]
    zt_k = [_tile([128, ZCOLS], BF16, name=f"zt{k}") for k in range(KT)]
    sq_k = [_tile([128, RPC], BF16, name=f"sq{k}") for k in range(KT)]
    supf_k = [_tile([128, NSUP], F32, name=f"supf{k}") for k in range(KT)]
    b1s = _tile([128, KT], F32, name="b1s")
    b2s = _tile([128, KT], F32, name="b2s")
    bbts = _tile([NC4, 1], F32, name="bbts")
    lamis = _tile([W1, 50], F32, name="lamis")
    coefs = _tile([128, 4], F32, name="coefs")
    e1s = _tile([SBP, W1], BF16, name="e1s")
    e2s = _tile([SBP, W1], BF16, name="e2s")
    eqs = _tile([128, W1], BF16, name="eqs")
    g_sb = _tile([SBP, SBP], F32, name="g_sb")
    gtmp = _tile([W1, 50], F32, name="gtmp")
    aug = _tile([W1, 50], F32, name="aug")
    augb = _tile([W1, 50], BF16, name="augb")
    minv = _tile([SBP, SBP], BF16, name="minv")
    at_sb = _tile([SBP, RPC], BF16, name="at_sb")
    bt_sb = _tile([SBP, RPC], BF16, name="bt_sb")
    v_sb = _tile([SBP, RPC], BF16, name="v_sb")
    p2_sb = _tile([SBP, RPC], BF16, name="p2_sb")
    sc_sb = _tile([W1, RPC], F32, name="sc_sb")
    bb_sb = _tile([NC4, RPC], F32, name="bb_sb")

    # DRAM scratch for the diagonal-block bounce
    dp = ctx.enter_context(tc.tile_pool(name="dscr", bufs=1, space="DRAM"))
    gd = dp.tile([SBP, SBP], F32, tag="gd", name="gd")
    md = dp.tile([SBP, SBP], BF16, tag="md", name="md")

    def xss(k):  # sup+bg k-tile slice
        return xts_a[:, k * NSB2 : (k + 1) * NSB2]

    def xsb(k):  # box k-tile slice
        return xtb_a[:, k * RPC : (k + 1) * RPC]

    def w1s(k, m):
        return w1_a[:, k * C + m * 128 : k * C + (m + 1) * 128]

    def w2s(k, m):
        return w2_a[:, k * C + m * 128 : k * C + (m + 1) * 128]

    def wbs(k):
        return wb_a[:, k * NC4 : (k + 1) * NC4]

    def load_all(dst, src, k0, k1, width):
        # dst[:, k*width+j] = src[k*128+p, j] for k in [k0,k1)
        d = dst[:, k0 * width : k1 * width].rearrange("p (k j) -> p k j", j=width)
        s = src[k0 * 128 : k1 * 128, :].rearrange("(k p) j -> p k j", p=128)
        nc.sync.dma_start(d, s)

    # ---- input DMAs: few big transfers, ordered to pace the compute ------
    load_all(xts_a, xts, 0, KT, NSB2)
    load_all(w1_a, w1t, 0, 4, C)
    load_all(w1_a, w1t, 4, KT, C)
    load_all(xtb_a, xtb, 0, KT, RPC)
    load_all(wb_a, wbt, 0, KT, NC4)
    nc.sync.dma_start(b1s[:], b1[:])
    nc.sync.dma_start(b2s[:], b2[:])
    nc.sync.dma_start(bbts[:], bbt[:])
    nc.sync.dma_start(lamis[:], lami[:])
    nc.sync.dma_start(coefs[:], coef[:])
    nc.sync.dma_start(e1s[:], e1[:])
    nc.sync.dma_start(e2s[:], e2[:])
    nc.sync.dma_start(eqs[:], eq[:])
    nc.gpsimd.dma_start(md[:], minz[:])  # zero the DRAM scatter scratch
    load_all(w2_a, w2t, 0, 4, C)
    load_all(w2_a, w2t, 4, KT, C)

    pp = ctx.enter_context(tc.tile_pool(name="psmm", bufs=5, space="PSUM"))
    sp = ctx.enter_context(tc.tile_pool(name="pssm", bufs=3, space="PSUM"))

    NSB = 2 * NSUP  # 200 sup+bg cols

    def l1_wave(wave, cs, n):
        ms = range(4 * wave, 4 * wave + 4)
        ps = {m: pp.tile([128, n], F32, tag="mm", name=f"l1ps{m}_{cs}") for m in ms}
        xsrc = xss if cs == 0 else xsb
        for k in range(KT):
            for m in ms:
                nc.tensor.matmul(
                    ps[m][:], w1s(k, m), xsrc(k),
                    start=(k == 0), stop=(k == KT - 1),
                )
        for m in ms:
            nc.scalar.add(ht_k[m][:, cs : cs + n], ps[m][:], b1s[:, m : m + 1])

    def l2_wave(wave, cs, n):
        ms = range(4 * wave, 4 * wave + 4)
        ps = {m: pp.tile([128, n], F32, tag="mm", name=f"l2ps{m}_{cs}") for m in ms}
        for k in range(KT):
            for m in ms:
                nc.tensor.matmul(
                    ps[m][:], w2s(k, m), ht_k[k][:, cs : cs + n],
                    start=(k == 0), stop=(k == KT - 1),
                )
        for m in ms:
            nc.scalar.add(
                zt_k[m][:, ZOFF + cs : ZOFF + cs + n], ps[m][:], b2s[:, m : m + 1]
            )
            if cs == 0:
                # f32 copy of the support rows straight from PSUM
                nc.scalar.add(supf_k[m][:], ps[m][:, 0:NSUP], b2s[:, m : m + 1])

    nc.vector.memset(gtmp[:], 0.0)

    # ---- AE layer 1 on sup+bg columns (early, to unblock the G/GJ chain) -
    l1_wave(0, 0, NSB)
    l1_wave(1, 0, NSB)

    # ---- bbox deltas: Wb @ box.T + b --------------------------------------
    ps_bb = pp.tile([NC4, RPC], F32, tag="mm", name="bbps")
    for k in range(KT):
        nc.tensor.matmul(
            ps_bb[:], wbs(k), xsb(k),
            start=(k == 0), stop=(k == KT - 1),
        )
    nc.scalar.add(bb_sb[:], ps_bb[:], bbts[:, 0:1])
    nc.sync.dma_start(bbox_t[:], bb_sb[:])

    # ---- AE layer 1 on box columns (overlaps the w2 DMA) ------------------
    l1_wave(0, NSB, RPC)
    l1_wave(1, NSB, RPC)

    # ---- AE layer 2 on sup+bg columns -> zt[:, 5:205] ---------------------
    l2_wave(0, 0, NSB)
    l2_wave(1, 0, NSB)

    # ---- support output + bg mean + gram G --------------------------------
    bmp = ctx.enter_context(tc.tile_pool(name="bmp", bufs=2))
    for k in range(KT):
        bm = bmp.tile([128, SHOT], F32, tag="bm")
        nc.vector.reduce_sum(
            bm[:],
            zt_k[k][:, ZOFF + NSUP : ZOFF + 2 * NSUP].rearrange(
                "p (w s) -> p s w", s=SHOT
            ),
            axis=mybir.AxisListType.X,
        )
        nc.vector.tensor_scalar_mul(zt_k[k][:, 0:SHOT], bm[:], 1.0 / WAY)

    ps_g = sp.tile([SBP, SBP], F32, tag="sm", name="gps")
    for k in range(KT):
        nc.tensor.matmul(
            ps_g[:], zt_k[k][:, 0:SBP], zt_k[k][:, 0:SBP],
            start=(k == 0), stop=(k == KT - 1),
        )
    nc.vector.tensor_copy(g_sb[:], ps_g[:])

    # ---- diagonal 5x5 blocks via DRAM bounce, add lam*I | I, Gauss-Jordan -
    # DRAM diag AP: block w at rows/cols [5w:5w+5] -> offset stride 5*112+5
    diag_g = AP(gd[:].tensor, 0, [[5 * SBP + 5, W1], [SBP, SHOT], [1, SHOT]])
    diag_m = AP(md[:].tensor, 0, [[5 * SBP + 5, W1], [SBP, SHOT], [1, SHOT]])
    g3 = gtmp[:].rearrange("p (r c) -> p r c", c=10)
    nc.sync.dma_start(gd[:], g_sb[:])
    nc.sync.dma_start(g3[:, :, 0:SHOT], diag_g)
    nc.vector.tensor_add(aug[:], gtmp[:], lamis[:])

    a3 = aug[:].rearrange("p (r c) -> p r c", c=10)
    ab3 = augb[:].rearrange("p (r c) -> p r c", c=10)
    gjp = ctx.enter_context(tc.tile_pool(name="gjp", bufs=2))
    for k in range(SHOT):
        # rows stay unnormalized: row_i -= (a_ik/a_kk) row_k for i != k
        piv = gjp.tile([W1, 1], F32, tag="piv")
        nc.vector.reciprocal(piv[:], a3[:, k, k : k + 1])
        negr = gjp.tile([W1, SHOT], F32, tag="negr")
        nc.vector.tensor_scalar(
            negr[:], a3[:, :, k], piv[:, 0:1], -1.0,
            mybir.AluOpType.mult, mybir.AluOpType.mult,
        )
        for i in range(SHOT):
            if i == k:
                continue
            nc.vector.scalar_tensor_tensor(
                a3[:, i, :],
                a3[:, k, :],
                negr[:, i : i + 1],
                a3[:, i, :],
                mybir.AluOpType.mult,
                mybir.AluOpType.add,
            )
    # left half is now diagonal; minv rows = right half rows / diag
    rdiag = gjp.tile([W1, SHOT], F32, tag="rdiag")
    nc.vector.reciprocal(rdiag[:], AP(aug[:].tensor, 0, [[50, W1], [11, SHOT]]))
    nc.vector.tensor_mul(
        ab3[:, :, SHOT:10],
        a3[:, :, SHOT:10],
        AP(rdiag[:].tensor, rdiag[:].offset, [[rdiag[:].ap[0][0], W1], [1, SHOT], [0, SHOT]]),
    )

    # ---- scatter m_inv into block-diagonal [112,112] via DRAM bounce -----
    nc.sync.dma_start(diag_m, ab3[:, :, SHOT:10])
    nc.sync.dma_start(minv[:], md[:])
    for k in range(KT):
        nc.gpsimd.dma_start(support_t[k * 128 : (k + 1) * 128, :], supf_k[k][:])

    # ---- L2 box columns, interleaved with squares / qnorm / A ------------
    ps_s = sp.tile([W1, RPC], F32, tag="sm", name="sps")
    ps_a = sp.tile([SBP, RPC], F32, tag="sm", name="aps")

    def sq_eq_a(ks):
        for k in ks:
            # sq = (q * sqrt(|c3|))^2 on the scalar engine; eqs holds -1
            nc.scalar.activation(
                sq_k[k][:],
                zt_k[k][:, ZBOX:ZCOLS],
                mybir.ActivationFunctionType.Square,
                bias=0.0,
                scale=coefs[:, 3:4],
            )
            nc.tensor.matmul(ps_s[:], eqs[:], sq_k[k][:], start=(k == 0), stop=False)
        for k in ks:
            nc.tensor.matmul(
                ps_a[:],
                zt_k[k][:, 0:SBP],
                zt_k[k][:, ZBOX:ZCOLS],
                start=(k == 0),
                stop=(k == KT - 1),
            )

    l2_wave(0, NSB, RPC)
    sq_eq_a(range(0, 4))
    l2_wave(1, NSB, RPC)
    sq_eq_a(range(4, KT))
    nc.vector.tensor_copy(at_sb[:], ps_a[:])

    # ---- B.T = Minv_big @ A.T --------------------------------------------
    ps_b = sp.tile([SBP, RPC], F32, tag="sm", name="bps")
    nc.tensor.matmul(ps_b[:], minv[:], at_sb[:], start=True, stop=True)
    nc.vector.tensor_copy(bt_sb[:], ps_b[:])

    # ---- scores: += E1.T@(c1*A*B) + E2.T@(c2*B*B) ------------------------
    nc.vector.scalar_tensor_tensor(
        v_sb[:], at_sb[:], coefs[:SBP, 0:1], bt_sb[:],
        mybir.AluOpType.mult, mybir.AluOpType.mult,
    )
    nc.vector.scalar_tensor_tensor(
        p2_sb[:], bt_sb[:], coefs[:SBP, 1:2], bt_sb[:],
        mybir.AluOpType.mult, mybir.AluOpType.mult,
    )
    nc.tensor.matmul(ps_s[:], e1s[:], v_sb[:], start=False, stop=False)
    nc.tensor.matmul(ps_s[:], e2s[:], p2_sb[:], start=False, stop=True)
    nc.vector.tensor_copy(sc_sb[:], ps_s[:])
    nc.sync.dma_start(scores_t[:], sc_sb[:])


def build():
    nc = bacc.Bacc("TRN2", target_bir_lowering=False, debug=False, num_devices=N_CORES)
    io = {}
    for name, shape, dt_ in [
        ("xts", (C, NSB2), BF16),
        ("xtb", (C, RPC), BF16),
        ("w1t", (C, C), BF16),
        ("w2t", (C, C), BF16),
        ("wbt", (C, NC4), BF16),
        ("b1", (128, KT), F32),
        ("b2", (128, KT), F32),
        ("bbt", (NC4, 1), F32),
        ("lami", (W1, 50), F32),
        ("coef", (128, 4), F32),
        ("e1", (SBP, W1), BF16),
        ("e2", (SBP, W1), BF16),
        ("minz", (SBP, SBP), BF16),
        ("eq", (128, W1), BF16),
    ]:
        io[name] = nc.dram_tensor(name, shape, dt_, kind="ExternalInput").ap()
    for name, shape, dt_ in [
        ("scores_t", (W1, RPC), F32),
        ("bbox_t", (NC4, RPC), F32),
        ("support_t", (C, NSUP), F32),
    ]:
        io[name] = nc.dram_tensor(name, shape, dt_, kind="ExternalOutput").ap()
    from contextlib import ExitStack

    with tile.TileContext(nc) as tc, ExitStack() as ctx:
        _emit(tc, nc, io, ctx)
    nc.compile()
    return nc


def host_inputs(support_fc, bg_fc, box_fc, W_ae1, b_ae1, W_ae2, b_ae2, W_bbox,
                b_bbox, r, scale):
    """Build the per-core input maps (all host-side layout prep)."""
    f = np.float32
    bf = ml_dtypes.bfloat16
    support_fc = np.asarray(support_fc, f)
    bg_fc = np.asarray(bg_fc, f)
    box_fc = np.asarray(box_fc, f)

    lam = f(SHOT) / f(C) * np.exp(f(r[0])) + f(1e-6)
    rho = np.exp(f(r[1]))
    es = np.exp(f(scale[0]))
    c1 = -es * (rho * rho - 2.0 * rho) / f(C)
    c2 = es * rho * rho * lam / f(C)
    c3 = -es / f(C)

    common = np.concatenate([support_fc, bg_fc], axis=0).T  # (C, 200)
    boxT = box_fc.T  # (C, 2048)

    w1t = np.ascontiguousarray(np.asarray(W_ae1, f).T.astype(bf))
    w2t = np.ascontiguousarray(np.asarray(W_ae2, f).T.astype(bf))
    wbt = np.ascontiguousarray(np.asarray(W_bbox, f).T.astype(bf))
    b1 = np.ascontiguousarray(np.asarray(b_ae1, f).reshape(KT, 128).T)
    b2 = np.ascontiguousarray(np.asarray(b_ae2, f).reshape(KT, 128).T)
    bbt = np.asarray(b_bbox, f).reshape(NC4, 1).copy()

    lami = np.zeros((W1, 50), f)
    for rr in range(SHOT):
        lami[:, rr * 10 + rr] = lam
        lami[:, rr * 10 + SHOT + rr] = 1.0
    coef = np.zeros((128, 4), f)
    coef[:, 0] = c1
    coef[:, 1] = c2
    coef[:, 2] = c3
    coef[:, 3] = np.sqrt(-c3)
    E = np.zeros((SBP, W1), f)
    for w in range(W1):
        E[5 * w : 5 * w + 5, w] = 1.0
    e1 = np.ascontiguousarray(E.astype(bf))
    e2 = np.ascontiguousarray(E.astype(bf))
    eq = np.full((128, W1), -1.0, bf)
    minz = np.zeros((SBP, SBP), bf)

    xts_h = np.ascontiguousarray(common.astype(bf))
    in_maps = []
    for c in range(N_CORES):
        xtb_h = np.ascontiguousarray(boxT[:, c * RPC : (c + 1) * RPC].astype(bf))
        in_maps.append(
            dict(xts=xts_h, xtb=xtb_h, w1t=w1t, w2t=w2t, wbt=wbt, b1=b1, b2=b2,
                 bbt=bbt, lami=lami, coef=coef, e1=e1, e2=e2, eq=eq, minz=minz)
        )
    return in_maps


def assemble(results):
    scores = np.empty((ROI, NCLS), np.float32)
    bbox = np.empty((ROI, NC4), np.float32)
    for c in range(N_CORES):
        scores[c * RPC : (c + 1) * RPC, :] = results[c]["scores_t"].T
        bbox[c * RPC : (c + 1) * RPC, :] = results[c]["bbox_t"].T
    support = (
        np.asarray(results[0]["support_t"], np.float32).T.reshape(WAY, SHOT, C).copy()
    )
    return scores, bbox, support


_NC = None


def kernel(support_fc, bg_fc, query_fc, box_fc, W_ae1, b_ae1, W_ae2, b_ae2,
           W_bbox, b_bbox, r, scale, **_unused):
    global _NC
    if _NC is None:
        _NC = build()
    in_maps = host_inputs(support_fc, bg_fc, box_fc, W_ae1, b_ae1, W_ae2,
                          b_ae2, W_bbox, b_bbox, r, scale)
    res = run_bass_kernel_spmd(_NC, in_maps, core_ids=list(range(N_CORES)))
    return assemble(res.results)


# revision 18
# speedup vs baseline: 1.0434x; 1.0434x over previous
"""Trainium2 Bass kernel for nn_FRPredictHeadWithFlatten.

Math restructuring (vs the reference):
  hat[w] = sb[w].T @ minv[w] @ sb[w]  (C x C) is never formed. With
  A = q @ sb.T (roi x 105, per-way 5-col blocks) and B[w] = A[w] @ minv[w]:
    Qbar.q = rho * rowsum(A*B),   |Qbar|^2 = rho^2*(dot - lam*rowsum(B*B))
  so scores = c1*rowsum(A*B) + c2*rowsum(B*B) + c3*|q|^2 with host scalars
  c1 = -exp(scale)*(rho^2-2rho)/C, c2 = exp(scale)*rho^2*lam/C,
  c3 = -exp(scale)/C (folded in as f32 per-partition scalars so they are
  never bf16-rounded; the selector matrices hold exact 0/1).
  The per-way row reductions run on the PE with selector matmuls so
  everything stays in transposed-activation layout. The 21 5x5 inverses are
  Gauss-Jordan in f32 on the vector engine in [way, 5*10] layout; the
  diagonal block extract/scatter goes through DRAM scratch (strided DRAM
  APs can address block diagonals in one DMA; SBUF APs cannot).

Sharding: data-parallel over roi (256 rows/core), weights + support/bg
replicated. All activations flow as X.T ([C on partitions, rows free]) so
every matmul has its contraction on partitions; weights are uploaded
pre-transposed and bf16-cast from the host (fp32 accumulate in PSUM; the
support output is additionally written back at f32 from PSUM).
"""

import sys

sys.path.insert(0, "/opt/trn_rl_repo")

import numpy as np
import ml_dtypes

import concourse.bacc as bacc
import concourse.tile as tile
from concourse.ap import AP
from concourse import mybir
from concourse.bass_utils import run_bass_kernel_spmd

WAY = 20
SHOT = 5
C = 1024
NCLS = 21
NC4 = NCLS * 4  # 84
ROI = 2048
N_CORES = 8
RPC = ROI // N_CORES  # 256 rois per core
W1 = WAY + 1  # 21 ways incl. background
SB = W1 * SHOT  # 105 sb rows
SBP = 112  # sb window incl. 7 junk cols (flow to zero through minv/E)
NSB2 = 200  # sup+bg columns
KT = C // 128  # 8 contraction tiles
NSUP = WAY * SHOT  # 100
NCOL = 2 * NSUP + RPC  # 456 act cols: [0:100 sup][100:200 bg][200:456 box]
# ZT col layout: [0:5 bgmean][5:105 s][105:205 bg][205:461 box]
ZOFF = 5
ZBOX = ZOFF + 2 * NSUP  # 205
ZCOLS = ZOFF + NCOL  # 461

F32 = mybir.dt.float32
BF16 = mybir.dt.bfloat16


def _emit(tc, nc, io, ctx):
    persist = ctx.enter_context(tc.tile_pool(name="persist", bufs=1))

    def _tile(shape, dtype, name):
        return persist.tile(shape, dtype, tag=name, name=name)

    xts, xtb, w1t, w2t, wbt = io["xts"], io["xtb"], io["w1t"], io["w2t"], io["wbt"]
    minz = io["minz"]
    b1, b2, bbt, lami, coef = io["b1"], io["b2"], io["bbt"], io["lami"], io["coef"]
    e1, e2, eq = io["e1"], io["e2"], io["eq"]
    scores_t, bbox_t, support_t = io["scores_t"], io["bbox_t"], io["support_t"]

    # ---- persistent SBUF tiles -------------------------------------------
    xts_a = _tile([128, KT * NSB2], BF16, name="xts_a")
    xtb_a = _tile([128, KT * RPC], BF16, name="xtb_a")
    w1_a = _tile([128, KT * C], BF16, name="w1_a")
    w2_a = _tile([128, KT * C], BF16, name="w2_a")
    wb_a = _tile([128, KT * NC4], BF16, name="wb_a")
    ht_k = [_tile([128, NCOL], BF16, name=f"ht{k}") for k in range(KT)]
    zt_k = [_tile([128, ZCOLS], BF16, name=f"zt{k}") for k in range(KT)]
    sq_k = [_tile([128, RPC], BF16, name=f"sq{k}") for k in range(KT)]
    supf_k = [_tile([128, NSUP], F32, name=f"supf{k}") for k in range(KT)]
    b1s = _tile([128, KT], F32, name="b1s")
    b2s = _tile([128, KT], F32, name="b2s")
    bbts = _tile([NC4, 1], F32, name="bbts")
    lamis = _tile([W1, 50], F32, name="lamis")
    coefs = _tile([128, 4], F32, name="coefs")
    e1s = _tile([SBP, W1], BF16, name="e1s")
    e2s = _tile([SBP, W1], BF16, name="e2s")
    eqs = _tile([128, W1], BF16, name="eqs")
    g_sb = _tile([SBP, SBP], F32, name="g_sb")
    gtmp = _tile([W1, 50], F32, name="gtmp")
    aug = _tile([W1, 50], F32, name="aug")
    augb = _tile([W1, 50], BF16, name="augb")
    minv = _tile([SBP, SBP], BF16, name="minv")
    at_sb = _tile([SBP, RPC], BF16, name="at_sb")
    bt_sb = _tile([SBP, RPC], BF16, name="bt_sb")
    v_sb = _tile([SBP, RPC], BF16, name="v_sb")
    p2_sb = _tile([SBP, RPC], BF16, name="p2_sb")
    sc_sb = _tile([W1, RPC], F32, name="sc_sb")
    bb_sb = _tile([NC4, RPC], F32, name="bb_sb")

    # DRAM scratch for the diagonal-block bounce
    dp = ctx.enter_context(tc.tile_pool(name="dscr", bufs=1, space="DRAM"))
    gd = dp.tile([SBP, SBP], F32, tag="gd", name="gd")
    md = dp.tile([SBP, SBP], BF16, tag="md", name="md")

    def xss(k):  # sup+bg k-tile slice
        return xts_a[:, k * NSB2 : (k + 1) * NSB2]

    def xsb(k):  # box k-tile slice
        return xtb_a[:, k * RPC : (k + 1) * RPC]

    def w1s(k, m):
        return w1_a[:, k * C + m * 128 : k * C + (m + 1) * 128]

    def w2s(k, m):
        return w2_a[:, k * C + m * 128 : k * C + (m + 1) * 128]

    def wbs(k):
        return wb_a[:, k * NC4 : (k + 1) * NC4]

    def load_all(dst, src, k0, k1, width):
        # dst[:, k*width+j] = src[k*128+p, j] for k in [k0,k1)
        d = dst[:, k0 * width : k1 * width].rearrange("p (k j) -> p k j", j=width)
        s = src[k0 * 128 : k1 * 128, :].rearrange("(k p) j -> p k j", p=128)
        nc.sync.dma_start(d, s)

    # ---- input DMAs: few big transfers, ordered to pace the compute ------
    load_all(xts_a, xts, 0, KT, NSB2)
    load_all(w1_a, w1t, 0, 4, C)
    load_all(w1_a, w1t, 4, KT, C)
    load_all(xtb_a, xtb, 0, KT, RPC)
    load_all(wb_a, wbt, 0, KT, NC4)
    nc.sync.dma_start(b1s[:], b1[:])
    nc.sync.dma_start(b2s[:], b2[:])
    nc.sync.dma_start(bbts[:], bbt[:])
    nc.sync.dma_start(lamis[:], lami[:])
    nc.sync.dma_start(coefs[:], coef[:])
    nc.sync.dma_start(e1s[:], e1[:])
    nc.sync.dma_start(e2s[:], e2[:])
    nc.sync.dma_start(eqs[:], eq[:])
    nc.gpsimd.dma_start(md[:], minz[:])  # zero the DRAM scatter scratch
    load_all(w2_a, w2t, 0, 4, C)
    load_all(w2_a, w2t, 4, KT, C)

    pp = ctx.enter_context(tc.tile_pool(name="psmm", bufs=5, space="PSUM"))
    sp = ctx.enter_context(tc.tile_pool(name="pssm", bufs=3, space="PSUM"))

    NSB = 2 * NSUP  # 200 sup+bg cols
    bmp = ctx.enter_context(tc.tile_pool(name="bmp", bufs=4))

    def l1_wave(wave, cs, n):
        ms = range(4 * wave, 4 * wave + 4)
        ps = {m: pp.tile([128, n], F32, tag="mm", name=f"l1ps{m}_{cs}") for m in ms}
        xsrc = xss if cs == 0 else xsb
        for k in range(KT):
            for m in ms:
                nc.tensor.matmul(
                    ps[m][:], w1s(k, m), xsrc(k),
                    start=(k == 0), stop=(k == KT - 1),
                )
        for m in ms:
            nc.scalar.add(ht_k[m][:, cs : cs + n], ps[m][:], b1s[:, m : m + 1])

    def l2_wave(wave, cs, n):
        ms = range(4 * wave, 4 * wave + 4)
        ps = {m: pp.tile([128, n], F32, tag="mm", name=f"l2ps{m}_{cs}") for m in ms}
        for k in range(KT):
            for m in ms:
                nc.tensor.matmul(
                    ps[m][:], w2s(k, m), ht_k[k][:, cs : cs + n],
                    start=(k == 0), stop=(k == KT - 1),
                )
        for m in ms:
            nc.scalar.add(
                zt_k[m][:, ZOFF + cs : ZOFF + cs + n], ps[m][:], b2s[:, m : m + 1]
            )
        if cs == 0:
            for m in ms:
                # bg mean straight from PSUM (pre-bias): mean = sum/20 + b2
                bm = bmp.tile([128, SHOT], F32, tag="bm", name=f"bm{m}")
                nc.vector.reduce_sum(
                    bm[:],
                    ps[m][:, NSUP : 2 * NSUP].rearrange("p (w s) -> p s w", s=SHOT),
                    axis=mybir.AxisListType.X,
                )
                nc.vector.tensor_scalar(
                    zt_k[m][:, 0:SHOT], bm[:], 1.0 / WAY, b2s[:, m : m + 1],
                    mybir.AluOpType.mult, mybir.AluOpType.add,
                )
            for m in ms:
                # f32 copy of the support rows straight from PSUM
                nc.scalar.add(supf_k[m][:], ps[m][:, 0:NSUP], b2s[:, m : m + 1])

    nc.vector.memset(gtmp[:], 0.0)

    # ---- AE layer 1 on sup+bg columns (early, to unblock the G/GJ chain) -
    l1_wave(0, 0, NSB)
    l1_wave(1, 0, NSB)

    # ---- bbox deltas: Wb @ box.T + b --------------------------------------
    ps_bb = pp.tile([NC4, RPC], F32, tag="mm", name="bbps")
    for k in range(KT):
        nc.tensor.matmul(
            ps_bb[:], wbs(k), xsb(k),
            start=(k == 0), stop=(k == KT - 1),
        )
    nc.scalar.add(bb_sb[:], ps_bb[:], bbts[:, 0:1])
    nc.sync.dma_start(bbox_t[:], bb_sb[:])

    # ---- AE layer 1 on box columns (overlaps the w2 DMA) ------------------
    l1_wave(0, NSB, RPC)
    l1_wave(1, NSB, RPC)

    # ---- AE layer 2 on sup+bg columns -> zt[:, 5:205] ---------------------
    l2_wave(0, 0, NSB)
    l2_wave(1, 0, NSB)

    # ---- support output + bg mean + gram G --------------------------------
    ps_g = sp.tile([SBP, SBP], F32, tag="sm", name="gps")
    for k in range(KT):
        nc.tensor.matmul(
            ps_g[:], zt_k[k][:, 0:SBP], zt_k[k][:, 0:SBP],
            start=(k == 0), stop=(k == KT - 1),
        )
    nc.vector.tensor_copy(g_sb[:], ps_g[:])

    # ---- diagonal 5x5 blocks via DRAM bounce, add lam*I | I, Gauss-Jordan -
    # DRAM diag AP: block w at rows/cols [5w:5w+5] -> offset stride 5*112+5
    diag_g = AP(gd[:].tensor, 0, [[5 * SBP + 5, W1], [SBP, SHOT], [1, SHOT]])
    diag_m = AP(md[:].tensor, 0, [[5 * SBP + 5, W1], [SBP, SHOT], [1, SHOT]])
    g3 = gtmp[:].rearrange("p (r c) -> p r c", c=10)
    nc.sync.dma_start(gd[:], g_sb[:])
    nc.sync.dma_start(g3[:, :, 0:SHOT], diag_g)
    nc.vector.tensor_add(aug[:], gtmp[:], lamis[:])

    a3 = aug[:].rearrange("p (r c) -> p r c", c=10)
    ab3 = augb[:].rearrange("p (r c) -> p r c", c=10)
    gjp = ctx.enter_context(tc.tile_pool(name="gjp", bufs=2))
    for k in range(SHOT):
        # rows stay unnormalized: row_i -= (a_ik/a_kk) row_k for i != k
        piv = gjp.tile([W1, 1], F32, tag="piv")
        nc.vector.reciprocal(piv[:], a3[:, k, k : k + 1])
        negr = gjp.tile([W1, SHOT], F32, tag="negr")
        nc.vector.tensor_scalar(
            negr[:], a3[:, :, k], piv[:, 0:1], -1.0,
            mybir.AluOpType.mult, mybir.AluOpType.mult,
        )
        for i in range(SHOT):
            if i == k:
                continue
            nc.vector.scalar_tensor_tensor(
                a3[:, i, :],
                a3[:, k, :],
                negr[:, i : i + 1],
                a3[:, i, :],
                mybir.AluOpType.mult,
                mybir.AluOpType.add,
            )
    # left half is now diagonal; minv rows = right half rows / diag
    rdiag = gjp.tile([W1, SHOT], F32, tag="rdiag")
    nc.vector.reciprocal(rdiag[:], AP(aug[:].tensor, 0, [[50, W1], [11, SHOT]]))
    nc.vector.tensor_mul(
        ab3[:, :, SHOT:10],
        a3[:, :, SHOT:10],
        AP(rdiag[:].tensor, rdiag[:].offset, [[rdiag[:].ap[0][0], W1], [1, SHOT], [0, SHOT]]),
    )

    # ---- scatter m_inv into block-diagonal [112,112] via DRAM bounce -----
    nc.sync.dma_start(diag_m, ab3[:, :, SHOT:10])
    nc.sync.dma_start(minv[:], md[:])
    for k in range(KT):
        nc.sync.dma_start(support_t[k * 128 : (k + 1) * 128, :], supf_k[k][:])

    # ---- L2 box columns, interleaved with squares / qnorm / A ------------
    ps_s = sp.tile([W1, RPC], F32, tag="sm", name="sps")
    ps_a = sp.tile([SBP, RPC], F32, tag="sm", name="aps")

    def sq_eq_a(ks):
        for k in ks:
            # sq = (q * sqrt(|c3|))^2 on the scalar engine; eqs holds -1
            nc.scalar.activation(
                sq_k[k][:],
                zt_k[k][:, ZBOX:ZCOLS],
                mybir.ActivationFunctionType.Square,
                bias=0.0,
                scale=coefs[:, 3:4],
            )
            nc.tensor.matmul(ps_s[:], eqs[:], sq_k[k][:], start=(k == 0), stop=False)
        for k in ks:
            nc.tensor.matmul(
                ps_a[:],
                zt_k[k][:, 0:SBP],
                zt_k[k][:, ZBOX:ZCOLS],
                start=(k == 0),
                stop=(k == KT - 1),
            )

    l2_wave(0, NSB, RPC)
    sq_eq_a(range(0, 4))
    l2_wave(1, NSB, RPC)
    sq_eq_a(range(4, KT))
    nc.scalar.copy(at_sb[:], ps_a[:])

    # ---- B.T = Minv_big @ A.T --------------------------------------------
    ps_b = sp.tile([SBP, RPC], F32, tag="sm", name="bps")
    nc.tensor.matmul(ps_b[:], minv[:], at_sb[:], start=True, stop=True)
    nc.scalar.copy(bt_sb[:], ps_b[:])

    # ---- scores: += E1.T@(c1*A*B) + E2.T@(c2*B*B) ------------------------
    nc.vector.scalar_tensor_tensor(
        v_sb[:], at_sb[:], coefs[:SBP, 0:1], bt_sb[:],
        mybir.AluOpType.mult, mybir.AluOpType.mult,
    )
    nc.vector.scalar_tensor_tensor(
        p2_sb[:], bt_sb[:], coefs[:SBP, 1:2], bt_sb[:],
        mybir.AluOpType.mult, mybir.AluOpType.mult,
    )
    nc.tensor.matmul(ps_s[:], e1s[:], v_sb[:], start=False, stop=False)
    nc.tensor.matmul(ps_s[:], e2s[:], p2_sb[:], start=False, stop=True)
    nc.scalar.copy(sc_sb[:], ps_s[:])
    nc.sync.dma_start(scores_t[:], sc_sb[:])


def build():
    nc = bacc.Bacc("TRN2", target_bir_lowering=False, debug=False, num_devices=N_CORES)
    io = {}
    for name, shape, dt_ in [
        ("xts", (C, NSB2), BF16),
        ("xtb", (C, RPC), BF16),
        ("w1t", (C, C), BF16),
        ("w2t", (C, C), BF16),
        ("wbt", (C, NC4), BF16),
        ("b1", (128, KT), F32),
        ("b2", (128, KT), F32),
        ("bbt", (NC4, 1), F32),
        ("lami", (W1, 50), F32),
        ("coef", (128, 4), F32),
        ("e1", (SBP, W1), BF16),
        ("e2", (SBP, W1), BF16),
        ("minz", (SBP, SBP), BF16),
        ("eq", (128, W1), BF16),
    ]:
        io[name] = nc.dram_tensor(name, shape, dt_, kind="ExternalInput").ap()
    for name, shape, dt_ in [
        ("scores_t", (W1, RPC), F32),
        ("bbox_t", (NC4, RPC), F32),
        ("support_t", (C, NSUP), F32),
    ]:
        io[name] = nc.dram_tensor(name, shape, dt_, kind="ExternalOutput").ap()
    from contextlib import ExitStack

    with tile.TileContext(nc) as tc, ExitStack() as ctx:
        _emit(tc, nc, io, ctx)
    nc.compile()
    return nc


def host_inputs(support_fc, bg_fc, box_fc, W_ae1, b_ae1, W_ae2, b_ae2, W_bbox,
                b_bbox, r, scale):
    """Build the per-core input maps (all host-side layout prep)."""
    f = np.float32
    bf = ml_dtypes.bfloat16
    support_fc = np.asarray(support_fc, f)
    bg_fc = np.asarray(bg_fc, f)
    box_fc = np.asarray(box_fc, f)

    lam = f(SHOT) / f(C) * np.exp(f(r[0])) + f(1e-6)
    rho = np.exp(f(r[1]))
    es = np.exp(f(scale[0]))
    c1 = -es * (rho * rho - 2.0 * rho) / f(C)
    c2 = es * rho * rho * lam / f(C)
    c3 = -es / f(C)

    common = np.concatenate([support_fc, bg_fc], axis=0).T  # (C, 200)
    boxT = box_fc.T  # (C, 2048)

    w1t = np.ascontiguousarray(np.asarray(W_ae1, f).T.astype(bf))
    w2t = np.ascontiguousarray(np.asarray(W_ae2, f).T.astype(bf))
    wbt = np.ascontiguousarray(np.asarray(W_bbox, f).T.astype(bf))
    b1 = np.ascontiguousarray(np.asarray(b_ae1, f).reshape(KT, 128).T)
    b2 = np.ascontiguousarray(np.asarray(b_ae2, f).reshape(KT, 128).T)
    bbt = np.asarray(b_bbox, f).reshape(NC4, 1).copy()

    lami = np.zeros((W1, 50), f)
    for rr in range(SHOT):
        lami[:, rr * 10 + rr] = lam
        lami[:, rr * 10 + SHOT + rr] = 1.0
    coef = np.zeros((128, 4), f)
    coef[:, 0] = c1
    coef[:, 1] = c2
    coef[:, 2] = c3
    coef[:, 3] = np.sqrt(-c3)
    E = np.zeros((SBP, W1), f)
    for w in range(W1):
        E[5 * w : 5 * w + 5, w] = 1.0
    e1 = np.ascontiguousarray(E.astype(bf))
    e2 = np.ascontiguousarray(E.astype(bf))
    eq = np.full((128, W1), -1.0, bf)
    minz = np.zeros((SBP, SBP), bf)

    xts_h = np.ascontiguousarray(common.astype(bf))
    in_maps = []
    for c in range(N_CORES):
        xtb_h = np.ascontiguousarray(boxT[:, c * RPC : (c + 1) * RPC].astype(bf))
        in_maps.append(
            dict(xts=xts_h, xtb=xtb_h, w1t=w1t, w2t=w2t, wbt=wbt, b1=b1, b2=b2,
                 bbt=bbt, lami=lami, coef=coef, e1=e1, e2=e2, eq=eq, minz=minz)
        )
    return in_maps


def assemble(results):
    scores = np.empty((ROI, NCLS), np.float32)
    bbox = np.empty((ROI, NC4), np.float32)
    for c in range(N_CORES):
        scores[c * RPC : (c + 1) * RPC, :] = results[c]["scores_t"].T
        bbox[c * RPC : (c + 1) * RPC, :] = results[c]["bbox_t"].T
    support = (
        np.asarray(results[0]["support_t"], np.float32).T.reshape(WAY, SHOT, C).copy()
    )
    return scores, bbox, support


_NC = None


def kernel(support_fc, bg_fc, query_fc, box_fc, W_ae1, b_ae1, W_ae2, b_ae2,
           W_bbox, b_bbox, r, scale, **_unused):
    global _NC
    if _NC is None:
        _NC = build()
    in_maps = host_inputs(support_fc, bg_fc, box_fc, W_ae1, b_ae1, W_ae2,
                          b_ae2, W_bbox, b_bbox, r, scale)
    res = run_bass_kernel_spmd(_NC, in_maps, core_ids=list(range(N_CORES)))
    return assemble(res.results)
